# revision 2
# baseline (speedup 1.0000x reference)
"""Trainium2 Bass kernel for nn_EnergyPredictor (segment-softmax attention
pooling) — v2: bf16 datapath.

Math per row set (rec nodes, lig nodes, cross edges): scores
s = relu(x@W1+b1)@W2+b2 (|s| < ~1 so softmax needs no max-subtraction),
then attention-pool per graph segment:

    pooled[g] = sum_{i in g} x_i * exp(s_i)  /  sum_{i in g} exp(s_i)

v2 layout (vs v1): x is DMA'd in bf16 [R, 256] (no ones column; halves HBM
traffic).  Per 128-row chunk the segment reduction runs TRANSPOSED with x
as the stationary operand:

    accT[fold_half k][128, 64] += x_chunk[:, k*128:(k+1)*128].T @ oh
    den[1, 64]                 += ones.T @ oh        (oh = onehot(idx)*exp(s))

which streams only 3*64 moving columns per 128 rows (vs 258).  The pooled
matrix is produced in transposed form [fold, G], which feeds the energy
MLP directly (no tail transposes).  Denominators ride in an extra DRAM row
of the all-reduce payload.

Sharding: rows split evenly over 8 cores; partial [fold, G] sums
all-reduced; the tiny 64-row energy MLP is replicated.
"""

import math
import numpy as np

P = 128            # partitions
G = 64             # num graphs
FOLD = 256
HALF = 128
NPG = 4            # 128-row chunks per group
RG = P * NPG       # rows per group = 512
NCORES = 8

_POOLS = ("rec", "lig", "cross")

_cache = {}

BUFS = dict(xs=10, xt=10, rh=8, small=8, pt=3, ph=2, ps=2)
RELU_ENGINE = "dve"   # "act" | "dve" | "both"
COPIES = "act"        # "split" | "act" | "dve"
SPLIT_AR = True
CC_SHARED = False
CC_BF16 = False
GPD = 1
OH_ENGINE = "dve"     # "dve" | "gpsimd"
TPOSE = "bf16"        # "bf16" | "packed" (f32r pair-packing: HW rounds, unsafe)


def _pad_rows(n_total):
    per = math.ceil(n_total / NCORES)
    return math.ceil(per / (GPD * RG)) * (GPD * RG)


def _build(cfg, repeat=1, collective=True, ncores=NCORES):
    """cfg: dict pool -> padded rows per core. Returns compiled nc."""
    import concourse.mybir as mybir
    import concourse.tile as tile
    from concourse import bacc
    from concourse.bass_interp import get_hw_module

    f32 = mybir.dt.float32
    bf16 = mybir.dt.bfloat16
    AF = mybir.ActivationFunctionType
    OP = mybir.AluOpType

    nc = bacc.Bacc("TRN2", target_bir_lowering=False, debug=False,
                   num_devices=ncores)

    # ---------------- DRAM I/O ----------------
    xd, idxd, w1d, w2d, b1d, b2d = {}, {}, {}, {}, {}, {}
    f32r_ = mybir.dt.float32r
    for pl in _POOLS:
        R = cfg[pl]
        if TPOSE == "packedr":
            xd[pl] = nc.dram_tensor(f"x_{pl}", [R, FOLD // 2], f32r_,
                                    kind="ExternalInput")
        else:
            xd[pl] = nc.dram_tensor(f"x_{pl}", [R, FOLD], bf16,
                                    kind="ExternalInput")
        idxd[pl] = nc.dram_tensor(f"idx_{pl}", [P, (R // RG) * NPG], f32,
                                  kind="ExternalInput")
        w1d[pl] = nc.dram_tensor(f"w1_{pl}", [FOLD, HALF], bf16, kind="ExternalInput")
        w2d[pl] = nc.dram_tensor(f"w2_{pl}", [HALF, 2], bf16, kind="ExternalInput")
        b1d[pl] = nc.dram_tensor(f"b1_{pl}", [HALF, 1], f32, kind="ExternalInput")
        b2d[pl] = nc.dram_tensor(f"b2_{pl}", [P, 1], f32, kind="ExternalInput")
    iotad = nc.dram_tensor("iota", [P, G], bf16, kind="ExternalInput")
    identd = nc.dram_tensor("ident", [P, P], bf16, kind="ExternalInput")
    identfd = nc.dram_tensor("identf", [P, P], mybir.dt.float32r, kind="ExternalInput")
    onesd = nc.dram_tensor("ones_col", [P, 2], bf16, kind="ExternalInput")
    ones1d = nc.dram_tensor("ones_row", [1, P], f32, kind="ExternalInput")
    mw1d = nc.dram_tensor("mlp_w1", [3 * FOLD, FOLD], f32, kind="ExternalInput")
    mb1d = nc.dram_tensor("mlp_b1", [P, 2], f32, kind="ExternalInput")
    mw2d = nc.dram_tensor("mlp_w2", [FOLD, HALF], f32, kind="ExternalInput")
    mb2d = nc.dram_tensor("mlp_b2", [P, 1], f32, kind="ExternalInput")
    owd = nc.dram_tensor("out_w", [HALF, 1], f32, kind="ExternalInput")
    obd = nc.dram_tensor("out_b", [1, 1], f32, kind="ExternalInput")
    energyd = nc.dram_tensor("energy", [G, 1], f32, kind="ExternalOutput")

    with tile.TileContext(nc) as tc:
        with (
            tc.tile_pool(name="const", bufs=1) as const,
            tc.tile_pool(name="xs", bufs=BUFS["xs"]) as xs_pool,
            tc.tile_pool(name="xt", bufs=BUFS["xt"]) as xt_pool,
            tc.tile_pool(name="rh", bufs=BUFS["rh"]) as rh_pool,
            tc.tile_pool(name="small", bufs=BUFS["small"]) as small_pool,
            tc.tile_pool(name="fin", bufs=2) as fin_pool,
            tc.tile_pool(name="psum_acc", bufs=1, space="PSUM") as psum_acc,
            tc.tile_pool(name="psum_t", bufs=BUFS["pt"], space="PSUM") as psum_t,
            tc.tile_pool(name="psum_h", bufs=BUFS["ph"], space="PSUM") as psum_h,
            tc.tile_pool(name="psum_s", bufs=BUFS["ps"], space="PSUM") as psum_s,
            tc.tile_pool(name="dram", bufs=1, space="DRAM") as dram_pool,
        ):
            # ---------------- constants / weights ----------------
            iota_sb = const.tile([P, G], bf16, tag="iota")
            nc.sync.dma_start(iota_sb[:], iotad.ap())
            ident_sb = const.tile([P, P], bf16, tag="ident")
            nc.sync.dma_start(ident_sb[:], identd.ap())
            identf_sb = const.tile([P, P], mybir.dt.float32r, tag="identf")
            nc.sync.dma_start(identf_sb[:], identfd.ap())
            ones_sb = const.tile([P, 2], bf16, tag="ones")
            nc.sync.dma_start(ones_sb[:], onesd.ap())
            ones1_sb = const.tile([1, P], f32, tag="ones1")
            nc.sync.dma_start(ones1_sb[:], ones1d.ap())

            w1_sb, w2_sb, b1_sb, b2_sb, idx_sb = {}, {}, {}, {}, {}
            for pl in _POOLS:
                w1_sb[pl] = const.tile([P, 2, HALF], bf16, tag=f"w1_{pl}",
                                       name=f"w1sb_{pl}")
                nc.sync.dma_start(
                    w1_sb[pl][:], w1d[pl].ap().rearrange("(c p) h -> p c h", p=P))
                w2_sb[pl] = const.tile([P, 2], bf16, tag=f"w2_{pl}",
                                       name=f"w2sb_{pl}")
                nc.sync.dma_start(w2_sb[pl][:], w2d[pl].ap())
                b1_sb[pl] = const.tile([HALF, 1], f32, tag=f"b1_{pl}",
                                       name=f"b1sb_{pl}")
                nc.sync.dma_start(b1_sb[pl][:], b1d[pl].ap())
                b2_sb[pl] = const.tile([P, 1], f32, tag=f"b2_{pl}",
                                       name=f"b2sb_{pl}")
                nc.sync.dma_start(b2_sb[pl][:], b2d[pl].ap())
                ncols = (cfg[pl] // RG) * NPG
                idx_sb[pl] = const.tile([P, ncols], f32, tag=f"idx_{pl}",
                                        name=f"idxsb_{pl}")
                nc.sync.dma_start(idx_sb[pl][:], idxd[pl].ap())

            mw1_sb = const.tile([P, 6, FOLD], f32, tag="mw1")
            nc.sync.dma_start(mw1_sb[:], mw1d.ap().rearrange("(k p) m -> p k m", p=P))
            mb1_sb = const.tile([P, 2], f32, tag="mb1")
            nc.sync.dma_start(mb1_sb[:], mb1d.ap())
            mw2_sb = const.tile([P, 2, HALF], f32, tag="mw2")
            nc.sync.dma_start(mw2_sb[:], mw2d.ap().rearrange("(m p) h -> p m h", p=P))
            mb2_sb = const.tile([P, 1], f32, tag="mb2")
            nc.sync.dma_start(mb2_sb[:], mb2d.ap())
            ow_sb = const.tile([P, 1], f32, tag="ow")
            nc.sync.dma_start(ow_sb[:], owd.ap())
            ob_sb = const.tile([1, 1], f32, tag="ob")
            nc.sync.dma_start(ob_sb[:], obd.ap())

            # ---------------- main loops ----------------
            for _rep in range(repeat):
                # partial sums, transposed: [fold-part, pool, half, G] + dens
                ccdt = bf16 if CC_BF16 else f32
                accsb = fin_pool.tile([P, 3, 2, G], ccdt, tag="accsb")
                dsb = fin_pool.tile([1, 3, G], ccdt, tag="dsb")

                for ipl, pl in enumerate(_POOLS):
                    accb = psum_acc.tile([P, 3 * G], f32, tag="acc",
                                         name=f"accps_{pl}")
                    acc0 = accb[:, 0:G]
                    acc1 = accb[:, G:2 * G]
                    den = accb[0:1, 2 * G:3 * G]
                    ngroups = cfg[pl] // RG
                    assert ngroups % GPD == 0
                    x_ap = xd[pl].ap().rearrange("(g n p) d -> g p n d",
                                                 n=NPG * GPD, p=P)
                    n_acc = ngroups * NPG
                    xs_macro = None
                    for g in range(ngroups):
                        if g % GPD == 0:
                            if TPOSE == "packedr":
                                xs_macro = xs_pool.tile(
                                    [P, NPG * GPD, FOLD // 2],
                                    mybir.dt.float32r, tag="xs", name="xs")
                            else:
                                xs_macro = xs_pool.tile([P, NPG * GPD, FOLD],
                                                        bf16, tag="xs", name="xs")
                            nc.sync.dma_start(xs_macro[:], x_ap[g // GPD])
                        xs = xs_macro[:, (g % GPD) * NPG:(g % GPD + 1) * NPG]

                        # transpose x -> xT [feat, rows]; one PSUM bank,
                        # one copy out.  "packed": f32r pair-transpose (bf16
                        # pairs ride in f32 lanes; W1 rows host-permuted).
                        if TPOSE in ("packed", "packedr"):
                            f32r = mybir.dt.float32r
                            tp = psum_t.tile([P, RG], f32r, tag="t")
                            for j in range(NPG):
                                nc.tensor.transpose(
                                    tp[:, j * P:(j + 1) * P],
                                    xs[:, j] if TPOSE == "packedr"
                                    else xs[:, j, :].bitcast(f32r),
                                    identf_sb[:])
                            xtp = xt_pool.tile([P, RG], f32r, tag="xt")
                            if COPIES == "act":
                                nc.scalar.activation(xtp[:], tp[:], AF.Copy)
                            elif COPIES == "split":
                                half = RG // 2
                                nc.scalar.activation(xtp[:, :half], tp[:, :half],
                                                     AF.Copy)
                                nc.vector.tensor_copy(xtp[:, half:], tp[:, half:])
                            else:
                                nc.vector.tensor_copy(xtp[:], tp[:])
                            xv = xtp[:].bitcast(bf16)
                            rhs2 = [xv[:, 0:2 * RG:2], xv[:, 1:2 * RG:2]]
                        else:
                            tp = psum_t.tile([P, 2, RG], bf16, tag="t")
                            for k in range(2):
                                for j in range(NPG):
                                    nc.tensor.transpose(
                                        tp[:, k, j * P:(j + 1) * P],
                                        xs[:, j, k * P:(k + 1) * P],
                                        ident_sb[:])
                            xt = xt_pool.tile([P, 2, RG], bf16, tag="xt")
                            if COPIES == "act":
                                nc.scalar.activation(xt[:], tp[:], AF.Copy)
                            elif COPIES == "split":
                                nc.scalar.activation(xt[:, 0], tp[:, 0], AF.Copy)
                                nc.vector.tensor_copy(xt[:, 1], tp[:, 1])
                            else:
                                nc.vector.tensor_copy(xt[:], tp[:])
                            rhs2 = [xt[:, 0], xt[:, 1]]

                        # hT [128, 512 rows] = W1.T @ xT  (accumulate 2 halves)
                        hp = psum_h.tile([P, RG], f32, tag="h")
                        for k in range(2):
                            nc.tensor.matmul(hp[:], w1_sb[pl][:, k, :], rhs2[k],
                                             start=(k == 0), stop=(k == 1))
                        rh = rh_pool.tile([P, RG], bf16, tag="rh")
                        if RELU_ENGINE == "dve":
                            nc.vector.tensor_scalar(rh[:], hp[:], b1_sb[pl][:], 0.0,
                                                    OP.add, OP.max)
                        elif RELU_ENGINE == "both":
                            half = RG // 2
                            nc.vector.tensor_scalar(rh[:, :half], hp[:, :half],
                                                    b1_sb[pl][:], 0.0,
                                                    OP.add, OP.max)
                            nc.scalar.activation(rh[:, half:], hp[:, half:],
                                                 AF.Relu, bias=b1_sb[pl][:])
                        else:
                            nc.scalar.activation(rh[:], hp[:], AF.Relu,
                                                 bias=b1_sb[pl][:])

                        # scores sT [128, 2] per chunk; exp on ACT
                        sp = psum_s.tile([P, NPG * 2], f32, tag="s")
                        for j in range(NPG):
                            nc.tensor.matmul(sp[:, 2 * j:2 * j + 2],
                                             rh[:, j * P:(j + 1) * P],
                                             w2_sb[pl][:],
                                             start=True, stop=True)
                        e = small_pool.tile([P, NPG * 2], f32, tag="e")
                        nc.scalar.activation(e[:], sp[:], AF.Exp,
                                             bias=b2_sb[pl][:])

                        # oh = onehot(idx) * e  [128 rows, G]
                        oh = small_pool.tile([P, NPG, G], bf16, tag="oh")
                        for j in range(NPG):
                            eng = nc.gpsimd if OH_ENGINE == "gpsimd" else nc.vector
                            eng.tensor_scalar(
                                oh[:, j], iota_sb[:],
                                idx_sb[pl][:, g * NPG + j:g * NPG + j + 1],
                                e[:, 2 * j:2 * j + 1], OP.is_equal, OP.mult)

                        # accT[k] += x_chunk[:, k].T @ oh ; den += ones.T @ oh
                        for j in range(NPG):
                            it = g * NPG + j
                            st, sp_ = (it == 0), (it == n_acc - 1)
                            if TPOSE == "packedr":
                                xsv = xs[:, j].bitcast(bf16)
                            else:
                                xsv = xs[:, j]
                            nc.tensor.matmul(acc0[:], xsv[:, 0:HALF],
                                             oh[:, j], start=st, stop=sp_,
                                             skip_group_check=not (st or sp_))
                            nc.tensor.matmul(acc1[:], xsv[:, HALF:FOLD],
                                             oh[:, j], start=False, stop=False,
                                             skip_group_check=True)
                            nc.tensor.matmul(den[:], ones_sb[:, 0:1],
                                             oh[:, j], start=False, stop=False,
                                             skip_group_check=True)

                    nc.vector.tensor_copy(accsb[:, ipl], accb[:, 0:2 * G])
                    nc.vector.tensor_copy(dsb[:, ipl], den[:])

                    if collective and SPLIT_AR and ipl == 1:
                        # reduce rec+lig now; overlaps the cross loop
                        cc1_in = dram_pool.tile([P + 1, 2 * 2 * G], ccdt,
                                                name="cc1_in")
                        cc1_out = dram_pool.tile([P + 1, 2 * 2 * G], ccdt,
                                                 name="cc1_out",
                                                 addr_space="Shared" if CC_SHARED else "Local")
                        nc.sync.dma_start(cc1_in[0:P, :], accsb[:, 0:2])
                        nc.sync.dma_start(cc1_in[P:P + 1, 0:2 * G], dsb[:, 0:2])
                        nc.gpsimd.collective_compute(
                            "AllReduce", mybir.AluOpType.add,
                            replica_groups=[list(range(ncores))],
                            ins=[cc1_in.opt()], outs=[cc1_out.opt()],
                        )

                # ---------------- all-reduce partials ----------------
                if collective and SPLIT_AR:
                    cc2_in = dram_pool.tile([P + 1, 2 * G], ccdt, name="cc2_in")
                    cc2_out = dram_pool.tile([P + 1, 2 * G], ccdt, name="cc2_out",
                                             addr_space="Shared" if CC_SHARED else "Local")
                    nc.sync.dma_start(cc2_in[0:P, :], accsb[:, 2])
                    nc.sync.dma_start(cc2_in[P:P + 1, 0:G], dsb[:, 2])
                    nc.gpsimd.collective_compute(
                        "AllReduce", mybir.AluOpType.add,
                        replica_groups=[list(range(ncores))],
                        ins=[cc2_in.opt()], outs=[cc2_out.opt()],
                    )
                    red = fin_pool.tile([P, 3, 2, G], ccdt, tag="red")
                    rden = fin_pool.tile([1, 3, G], ccdt, tag="rden")
                    nc.sync.dma_start(red[:, 0:2], cc1_out[0:P, :])
                    nc.sync.dma_start(rden[:, 0:2], cc1_out[P:P + 1, 0:2 * G])
                    nc.sync.dma_start(red[:, 2], cc2_out[0:P, :])
                    nc.sync.dma_start(rden[:, 2], cc2_out[P:P + 1, 0:G])
                elif collective:
                    cc_in = dram_pool.tile([P + 1, 3 * 2 * G], ccdt)
                    cc_out = dram_pool.tile([P + 1, 3 * 2 * G], ccdt,
                                            addr_space="Shared" if CC_SHARED else "Local")
                    nc.sync.dma_start(cc_in[0:P, :], accsb[:])
                    nc.sync.dma_start(cc_in[P:P + 1, 0:3 * G], dsb[:])
                    nc.gpsimd.collective_compute(
                        "AllReduce", mybir.AluOpType.add,
                        replica_groups=[list(range(ncores))],
                        ins=[cc_in.opt()], outs=[cc_out.opt()],
                    )
                    red = fin_pool.tile([P, 3, 2, G], ccdt, tag="red")
                    rden = fin_pool.tile([1, 3, G], ccdt, tag="rden")
                    nc.sync.dma_start(red[:], cc_out[0:P, :])
                    nc.sync.dma_start(rden[:], cc_out[P:P + 1, 0:3 * G])
                else:
                    red = accsb
                    rden = dsb

                # ---------------- pooledT = accT / den ; combT [128, 6, G] ----
                dsum = fin_pool.tile([1, 3, G], f32, tag="dsum")
                nc.vector.tensor_scalar(dsum[:], rden[:], 1e-30, None, OP.add)
                rcp = fin_pool.tile([1, 3, G], f32, tag="rcp")
                nc.vector.reciprocal(rcp[:], dsum[:])
                # broadcast rcp along partitions via K=1 matmul
                rcpb = psum_s.tile([P, 3 * G], f32, tag="s", name="rcpb")
                nc.tensor.matmul(rcpb[:], ones1_sb[:], rcp[0:1, :, :],
                                 start=True, stop=True)
                combT = fin_pool.tile([P, 6, G], f32, tag="combT")
                for k in range(6):
                    nc.vector.tensor_tensor(
                        combT[:, k], red[:, k // 2, k % 2],
                        rcpb[:, (k // 2) * G:(k // 2 + 1) * G], OP.mult)

                # ---------------- energy MLP (fp32, replicated) ----------------
                r1 = fin_pool.tile([P, 2, G], f32, tag="r1")
                for m in range(2):
                    h1p = psum_h.tile([P, G], f32, tag="h")
                    for k in range(6):
                        nc.tensor.matmul(h1p[:], mw1_sb[:, k, m * P:(m + 1) * P],
                                         combT[:, k], start=(k == 0), stop=(k == 5))
                    nc.scalar.activation(r1[:, m], h1p[:], AF.Relu,
                                         bias=mb1_sb[:, m:m + 1])
                h2p = psum_h.tile([P, G], f32, tag="h")
                for m in range(2):
                    nc.tensor.matmul(h2p[:], mw2_sb[:, m], r1[:, m],
                                     start=(m == 0), stop=(m == 1))
                r2 = fin_pool.tile([P, G], f32, tag="r2")
                nc.scalar.activation(r2[:], h2p[:], AF.Relu, bias=mb2_sb[:])

                ep = psum_s.tile([1, G], f32, tag="s", name="ep")
                nc.tensor.matmul(ep[:], ow_sb[:], r2[:], start=True, stop=True)
                en = fin_pool.tile([1, G], f32, tag="en")
                nc.vector.tensor_scalar(en[:], ep[:], ob_sb[:], None, OP.add)
                nc.sync.dma_start(energyd.ap(), en[:])

    nc.compile()
    nc.m = get_hw_module(nc.m)
    return nc


def _prep_pool(x, idx, n_pad):
    """Shard rows of x/idx across cores, pad to n_pad rows per core.
    Returns per-core lists (x [n_pad,256] bf16, idx [128, n_pad//128] bf16)."""
    import ml_dtypes
    n = x.shape[0]
    per = math.ceil(n / NCORES)
    xs, idxs = [], []
    for c in range(NCORES):
        lo, hi = c * per, min((c + 1) * per, n)
        xp = np.zeros((n_pad, FOLD), dtype=ml_dtypes.bfloat16)
        xp[:hi - lo] = x[lo:hi].astype(ml_dtypes.bfloat16)
        ip = np.full((n_pad,), -1.0, dtype=np.float32)
        ip[:hi - lo] = idx[lo:hi]
        # rearrange (g n p) -> p (g n)
        ip = ip.reshape(n_pad // RG, NPG, P).transpose(2, 0, 1)
        xs.append(xp)
        idxs.append(np.ascontiguousarray(ip.reshape(P, -1)))
    return xs, idxs


def kernel(rec_na, lig_na, cross_ea, cross_idx, protein_batch, ligand_batch,
           num_graphs,
           pa_W1, pa_b1, pa_W2, pa_b2,
           la_W1, la_b1, la_W2, la_b2,
           ca_W1, ca_b1, ca_W2, ca_b2,
           mlp_W1, mlp_b1, mlp_W2, mlp_b2,
           out_W, out_b):
    import ml_dtypes
    from concourse import bass_utils

    bf = ml_dtypes.bfloat16
    assert int(num_graphs) == G
    rec_na = np.asarray(rec_na, dtype=np.float32)
    lig_na = np.asarray(lig_na, dtype=np.float32)
    cross_ea = np.asarray(cross_ea, dtype=np.float32)
    cross_idx = np.asarray(cross_idx)
    protein_batch = np.asarray(protein_batch)
    ligand_batch = np.asarray(ligand_batch)
    complex_ids = ligand_batch[cross_idx[0]]

    data = {
        "rec": (rec_na, protein_batch.astype(np.float32),
                pa_W1, pa_b1, pa_W2, pa_b2),
        "lig": (lig_na, ligand_batch.astype(np.float32),
                la_W1, la_b1, la_W2, la_b2),
        "cross": (cross_ea, complex_ids.astype(np.float32),
                  ca_W1, ca_b1, ca_W2, ca_b2),
    }
    cfg = {pl: _pad_rows(data[pl][0].shape[0]) for pl in _POOLS}

    key = tuple(sorted(cfg.items()))
    if key not in _cache:
        _cache[key] = _build(cfg)
    nc = _cache[key]

    shared = {
        "iota": np.broadcast_to(np.arange(G, dtype=np.float32),
                                (P, G)).astype(bf),
        "ident": np.eye(P, dtype=np.float32).astype(bf),
        "identf": np.eye(P, dtype=np.float32),
        "ones_col": np.ones((P, 2), dtype=np.float32).astype(bf),
        "ones_row": np.ones((1, P), dtype=np.float32),
        "mlp_w1": np.asarray(mlp_W1, dtype=np.float32),
        "mlp_b1": np.ascontiguousarray(
            np.asarray(mlp_b1, dtype=np.float32).reshape(2, P).T),
        "mlp_w2": np.asarray(mlp_W2, dtype=np.float32),
        "mlp_b2": np.asarray(mlp_b2, dtype=np.float32).reshape(P, 1),
        "out_w": np.asarray(out_W, dtype=np.float32),
        "out_b": np.asarray(out_b, dtype=np.float32).reshape(1, 1),
    }
    percore = [dict(shared) for _ in range(NCORES)]
    for pl in _POOLS:
        x, idx, W1, b1, W2, b2 = data[pl]
        xs, idxs = _prep_pool(x, idx, cfg[pl])
        W1 = np.asarray(W1, dtype=np.float32)
        if TPOSE in ("packed", "packedr"):
            W1 = np.concatenate([W1[0::2], W1[1::2]])
        W1 = W1.astype(bf)
        W2 = np.concatenate([np.asarray(W2, np.float32).reshape(HALF, 1),
                             np.zeros((HALF, 1), np.float32)],
                            axis=1).astype(bf)
        b1 = np.asarray(b1, dtype=np.float32).reshape(HALF, 1)
        b2 = np.broadcast_to(np.asarray(b2, dtype=np.float32).reshape(1, 1),
                             (P, 1)).copy()
        for c in range(NCORES):
            percore[c][f"x_{pl}"] = (xs[c].view(np.float32)
                                     if TPOSE == "packedr" else xs[c])
            percore[c][f"idx_{pl}"] = idxs[c]
            percore[c][f"w1_{pl}"] = W1
            percore[c][f"w2_{pl}"] = W2
            percore[c][f"b1_{pl}"] = b1
            percore[c][f"b2_{pl}"] = b2

    global _LAST
    _LAST = (nc, percore)
    if _PREP_ONLY:
        return None
    res = bass_utils.run_bass_kernel_spmd(nc, percore,
                                          core_ids=list(range(NCORES)))
    return np.asarray(res.results[0]["energy"], dtype=np.float32)


def prepare(inputs):
    """Build (or fetch cached) program + per-core input maps without running."""
    global _PREP_ONLY
    _PREP_ONLY = True
    try:
        kernel(**inputs)
    finally:
        _PREP_ONLY = False
    return _LAST


_PREP_ONLY = False
_LAST = None


# revision 3
# speedup vs baseline: 93.0057x; 93.0057x over previous
"""Trainium2 Bass kernel for nn_EnergyPredictor (segment-softmax attention
pooling) — v2: bf16 datapath.

Math per row set (rec nodes, lig nodes, cross edges): scores
s = relu(x@W1+b1)@W2+b2 (|s| < ~1 so softmax needs no max-subtraction),
then attention-pool per graph segment:

    pooled[g] = sum_{i in g} x_i * exp(s_i)  /  sum_{i in g} exp(s_i)

v2 layout (vs v1): x is DMA'd in bf16 [R, 256] (no ones column; halves HBM
traffic).  Per 128-row chunk the segment reduction runs TRANSPOSED with x
as the stationary operand:

    accT[fold_half k][128, 64] += x_chunk[:, k*128:(k+1)*128].T @ oh
    den[1, 64]                 += ones.T @ oh        (oh = onehot(idx)*exp(s))

which streams only 3*64 moving columns per 128 rows (vs 258).  The pooled
matrix is produced in transposed form [fold, G], which feeds the energy
MLP directly (no tail transposes).  Denominators ride in an extra DRAM row
of the all-reduce payload.

Sharding: rows split evenly over 8 cores; partial [fold, G] sums
all-reduced; the tiny 64-row energy MLP is replicated.
"""

import math
import numpy as np

P = 128            # partitions
G = 64             # num graphs
FOLD = 256
HALF = 128
NPG = 4            # 128-row chunks per group
RG = P * NPG       # rows per group = 512
NCORES = 8

_POOLS = ("rec", "lig", "cross")

_cache = {}

BUFS = dict(xs=10, xt=10, rh=8, small=8, pt=3, ph=2, ps=2)
RELU_ENGINE = "dve"   # "act" | "dve" | "both"
COPIES = "act"        # "split" | "act" | "dve"
SPLIT_AR = True
CC_SHARED = False
CC_BF16 = False
GPD = 1
OH_ENGINE = "dve"     # "dve" | "gpsimd"
TPOSE = "bf16"        # "bf16" | "packed" (f32r pair-packing: HW rounds, unsafe)


def _pad_rows(n_total):
    per = math.ceil(n_total / NCORES)
    return math.ceil(per / (GPD * RG)) * (GPD * RG)


def _build(cfg, repeat=1, collective=True, ncores=NCORES):
    """cfg: dict pool -> padded rows per core. Returns compiled nc."""
    import concourse.mybir as mybir
    import concourse.tile as tile
    from concourse import bacc
    from concourse.bass_interp import get_hw_module

    f32 = mybir.dt.float32
    bf16 = mybir.dt.bfloat16
    AF = mybir.ActivationFunctionType
    OP = mybir.AluOpType

    nc = bacc.Bacc("TRN2", target_bir_lowering=False, debug=False,
                   num_devices=ncores)

    # ---------------- DRAM I/O ----------------
    xd, idxd, w1d, w2d, b1d, b2d = {}, {}, {}, {}, {}, {}
    f32r_ = mybir.dt.float32r
    for pl in _POOLS:
        R = cfg[pl]
        if TPOSE == "packedr":
            xd[pl] = nc.dram_tensor(f"x_{pl}", [R, FOLD // 2], f32r_,
                                    kind="ExternalInput")
        else:
            xd[pl] = nc.dram_tensor(f"x_{pl}", [R, FOLD], bf16,
                                    kind="ExternalInput")
        idxd[pl] = nc.dram_tensor(f"idx_{pl}", [P, (R // RG) * NPG], f32,
                                  kind="ExternalInput")
        w1d[pl] = nc.dram_tensor(f"w1_{pl}", [FOLD, HALF], bf16, kind="ExternalInput")
        w2d[pl] = nc.dram_tensor(f"w2_{pl}", [HALF, 2], bf16, kind="ExternalInput")
        b1d[pl] = nc.dram_tensor(f"b1_{pl}", [HALF, 1], f32, kind="ExternalInput")
        b2d[pl] = nc.dram_tensor(f"b2_{pl}", [P, 1], f32, kind="ExternalInput")
    iotad = nc.dram_tensor("iota", [P, G], bf16, kind="ExternalInput")
    identd = nc.dram_tensor("ident", [P, P], bf16, kind="ExternalInput")
    identfd = nc.dram_tensor("identf", [P, P], mybir.dt.float32r, kind="ExternalInput")
    onesd = nc.dram_tensor("ones_col", [P, 2], bf16, kind="ExternalInput")
    ones1d = nc.dram_tensor("ones_row", [1, P], f32, kind="ExternalInput")
    mw1d = nc.dram_tensor("mlp_w1", [3 * FOLD, FOLD], f32, kind="ExternalInput")
    mb1d = nc.dram_tensor("mlp_b1", [P, 2], f32, kind="ExternalInput")
    mw2d = nc.dram_tensor("mlp_w2", [FOLD, HALF], f32, kind="ExternalInput")
    mb2d = nc.dram_tensor("mlp_b2", [P, 1], f32, kind="ExternalInput")
    owd = nc.dram_tensor("out_w", [HALF, 1], f32, kind="ExternalInput")
    obd = nc.dram_tensor("out_b", [1, 1], f32, kind="ExternalInput")
    energyd = nc.dram_tensor("energy", [G, 1], f32, kind="ExternalOutput")

    with tile.TileContext(nc) as tc:
        with (
            tc.tile_pool(name="const", bufs=1) as const,
            tc.tile_pool(name="xs", bufs=BUFS["xs"]) as xs_pool,
            tc.tile_pool(name="xt", bufs=BUFS["xt"]) as xt_pool,
            tc.tile_pool(name="rh", bufs=BUFS["rh"]) as rh_pool,
            tc.tile_pool(name="small", bufs=BUFS["small"]) as small_pool,
            tc.tile_pool(name="fin", bufs=2) as fin_pool,
            tc.tile_pool(name="psum_acc", bufs=1, space="PSUM") as psum_acc,
            tc.tile_pool(name="psum_t", bufs=BUFS["pt"], space="PSUM") as psum_t,
            tc.tile_pool(name="psum_h", bufs=BUFS["ph"], space="PSUM") as psum_h,
            tc.tile_pool(name="psum_s", bufs=BUFS["ps"], space="PSUM") as psum_s,
            tc.tile_pool(name="dram", bufs=1, space="DRAM") as dram_pool,
        ):
            # ---------------- constants / weights ----------------
            iota_sb = const.tile([P, G], bf16, tag="iota")
            nc.sync.dma_start(iota_sb[:], iotad.ap())
            ident_sb = const.tile([P, P], bf16, tag="ident")
            nc.sync.dma_start(ident_sb[:], identd.ap())
            identf_sb = const.tile([P, P], mybir.dt.float32r, tag="identf")
            nc.sync.dma_start(identf_sb[:], identfd.ap())
            ones_sb = const.tile([P, 2], bf16, tag="ones")
            nc.sync.dma_start(ones_sb[:], onesd.ap())
            ones1_sb = const.tile([1, P], f32, tag="ones1")
            nc.sync.dma_start(ones1_sb[:], ones1d.ap())

            w1_sb, w2_sb, b1_sb, b2_sb, idx_sb = {}, {}, {}, {}, {}
            for pl in _POOLS:
                w1_sb[pl] = const.tile([P, 2, HALF], bf16, tag=f"w1_{pl}",
                                       name=f"w1sb_{pl}")
                nc.sync.dma_start(
                    w1_sb[pl][:], w1d[pl].ap().rearrange("(c p) h -> p c h", p=P))
                w2_sb[pl] = const.tile([P, 2], bf16, tag=f"w2_{pl}",
                                       name=f"w2sb_{pl}")
                nc.sync.dma_start(w2_sb[pl][:], w2d[pl].ap())
                b1_sb[pl] = const.tile([HALF, 1], f32, tag=f"b1_{pl}",
                                       name=f"b1sb_{pl}")
                nc.sync.dma_start(b1_sb[pl][:], b1d[pl].ap())
                b2_sb[pl] = const.tile([P, 1], f32, tag=f"b2_{pl}",
                                       name=f"b2sb_{pl}")
                nc.sync.dma_start(b2_sb[pl][:], b2d[pl].ap())
                ncols = (cfg[pl] // RG) * NPG
                idx_sb[pl] = const.tile([P, ncols], f32, tag=f"idx_{pl}",
                                        name=f"idxsb_{pl}")
                nc.sync.dma_start(idx_sb[pl][:], idxd[pl].ap())

            mw1_sb = const.tile([P, 6, FOLD], f32, tag="mw1")
            nc.sync.dma_start(mw1_sb[:], mw1d.ap().rearrange("(k p) m -> p k m", p=P))
            mb1_sb = const.tile([P, 2], f32, tag="mb1")
            nc.sync.dma_start(mb1_sb[:], mb1d.ap())
            mw2_sb = const.tile([P, 2, HALF], f32, tag="mw2")
            nc.sync.dma_start(mw2_sb[:], mw2d.ap().rearrange("(m p) h -> p m h", p=P))
            mb2_sb = const.tile([P, 1], f32, tag="mb2")
            nc.sync.dma_start(mb2_sb[:], mb2d.ap())
            ow_sb = const.tile([P, 1], f32, tag="ow")
            nc.sync.dma_start(ow_sb[:], owd.ap())
            ob_sb = const.tile([1, 1], f32, tag="ob")
            nc.sync.dma_start(ob_sb[:], obd.ap())

            # ---------------- main loops ----------------
            for _rep in range(repeat):
                # partial sums, transposed: [fold-part, pool, half, G] + dens
                ccdt = bf16 if CC_BF16 else f32
                accsb = fin_pool.tile([P, 3, 2, G], ccdt, tag="accsb")
                dsb = fin_pool.tile([1, 3, G], ccdt, tag="dsb")

                for ipl, pl in enumerate(_POOLS):
                    accb = psum_acc.tile([P, 6 * G], f32, tag="acc",
                                         name=f"accps_{pl}")
                    acc0 = accb[:, 0:G]
                    acc1 = accb[:, G:2 * G]
                    den = accb[0:1, 2 * G:6 * G]
                    ngroups = cfg[pl] // RG
                    assert ngroups % GPD == 0
                    x_ap = xd[pl].ap().rearrange("(g n p) d -> g p n d",
                                                 n=NPG * GPD, p=P)
                    n_acc = ngroups * NPG
                    xs_macro = None
                    for g in range(ngroups):
                        if g % GPD == 0:
                            if TPOSE == "packedr":
                                xs_macro = xs_pool.tile(
                                    [P, NPG * GPD, FOLD // 2],
                                    mybir.dt.float32r, tag="xs", name="xs")
                            else:
                                xs_macro = xs_pool.tile([P, NPG * GPD, FOLD],
                                                        bf16, tag="xs", name="xs")
                            nc.sync.dma_start(xs_macro[:], x_ap[g // GPD])
                        xs = xs_macro[:, (g % GPD) * NPG:(g % GPD + 1) * NPG]

                        # transpose x -> xT [feat, rows]; one PSUM bank,
                        # one copy out.  "packed": f32r pair-transpose (bf16
                        # pairs ride in f32 lanes; W1 rows host-permuted).
                        if TPOSE in ("packed", "packedr"):
                            f32r = mybir.dt.float32r
                            tp = psum_t.tile([P, RG], f32r, tag="t")
                            for j in range(NPG):
                                nc.tensor.transpose(
                                    tp[:, j * P:(j + 1) * P],
                                    xs[:, j] if TPOSE == "packedr"
                                    else xs[:, j, :].bitcast(f32r),
                                    identf_sb[:])
                            xtp = xt_pool.tile([P, RG], f32r, tag="xt")
                            if COPIES == "act":
                                nc.scalar.activation(xtp[:], tp[:], AF.Copy)
                            elif COPIES == "split":
                                half = RG // 2
                                nc.scalar.activation(xtp[:, :half], tp[:, :half],
                                                     AF.Copy)
                                nc.vector.tensor_copy(xtp[:, half:], tp[:, half:])
                            else:
                                nc.vector.tensor_copy(xtp[:], tp[:])
                            xv = xtp[:].bitcast(bf16)
                            rhs2 = [xv[:, 0:2 * RG:2], xv[:, 1:2 * RG:2]]
                        else:
                            tp = psum_t.tile([P, 2, RG], bf16, tag="t")
                            for k in range(2):
                                for j in range(NPG):
                                    nc.tensor.transpose(
                                        tp[:, k, j * P:(j + 1) * P],
                                        xs[:, j, k * P:(k + 1) * P],
                                        ident_sb[:])
                            xt = xt_pool.tile([P, 2, RG], bf16, tag="xt")
                            if COPIES == "act":
                                nc.scalar.activation(xt[:], tp[:], AF.Copy)
                            elif COPIES == "split":
                                nc.scalar.activation(xt[:, 0], tp[:, 0], AF.Copy)
                                nc.vector.tensor_copy(xt[:, 1], tp[:, 1])
                            else:
                                nc.vector.tensor_copy(xt[:], tp[:])
                            rhs2 = [xt[:, 0], xt[:, 1]]

                        # hT [128, 512 rows] = W1.T @ xT  (accumulate 2 halves)
                        hp = psum_h.tile([P, RG], f32, tag="h")
                        for k in range(2):
                            nc.tensor.matmul(hp[:], w1_sb[pl][:, k, :], rhs2[k],
                                             start=(k == 0), stop=(k == 1))
                        rh = rh_pool.tile([P, RG], bf16, tag="rh")
                        if RELU_ENGINE == "dve":
                            nc.vector.tensor_scalar(rh[:], hp[:], b1_sb[pl][:], 0.0,
                                                    OP.add, OP.max)
                        elif RELU_ENGINE == "both":
                            half = RG // 2
                            nc.vector.tensor_scalar(rh[:, :half], hp[:, :half],
                                                    b1_sb[pl][:], 0.0,
                                                    OP.add, OP.max)
                            nc.scalar.activation(rh[:, half:], hp[:, half:],
                                                 AF.Relu, bias=b1_sb[pl][:])
                        else:
                            nc.scalar.activation(rh[:], hp[:], AF.Relu,
                                                 bias=b1_sb[pl][:])

                        # scores sT [128, 2] per chunk; exp on ACT
                        sp = psum_s.tile([P, NPG * 2], f32, tag="s")
                        for j in range(NPG):
                            nc.tensor.matmul(sp[:, 2 * j:2 * j + 2],
                                             rh[:, j * P:(j + 1) * P],
                                             w2_sb[pl][:],
                                             start=True, stop=True)
                        e = small_pool.tile([P, NPG * 2], f32, tag="e")
                        nc.scalar.activation(e[:], sp[:], AF.Exp,
                                             bias=b2_sb[pl][:])

                        # oh = onehot(idx) * e  [128 rows, G]
                        oh = small_pool.tile([P, NPG, G], bf16, tag="oh")
                        for j in range(NPG):
                            eng = nc.gpsimd if OH_ENGINE == "gpsimd" else nc.vector
                            eng.tensor_scalar(
                                oh[:, j], iota_sb[:],
                                idx_sb[pl][:, g * NPG + j:g * NPG + j + 1],
                                e[:, 2 * j:2 * j + 1], OP.is_equal, OP.mult)

                        # accT[k] += x_chunk[:, k].T @ oh ; den += ones.T @ oh
                        for j in range(NPG):
                            it = g * NPG + j
                            st, sp_ = (it == 0), (it == n_acc - 1)
                            if TPOSE == "packedr":
                                xsv = xs[:, j].bitcast(bf16)
                            else:
                                xsv = xs[:, j]
                            nc.tensor.matmul(acc0[:], xsv[:, 0:HALF],
                                             oh[:, j], start=st, stop=sp_,
                                             skip_group_check=not (st or sp_))
                            nc.tensor.matmul(acc1[:], xsv[:, HALF:FOLD],
                                             oh[:, j], start=False, stop=False,
                                             skip_group_check=True)
                        # denominator: one matmul over all NPG chunks' oh
                        nc.tensor.matmul(den[:], ones_sb[:, 0:1],
                                         oh[:, :, :], start=False, stop=False,
                                         skip_group_check=True)

                    nc.vector.tensor_copy(accsb[:, ipl], accb[:, 0:2 * G])
                    dw = fin_pool.tile([1, 4, G], f32, tag="dw")
                    nc.vector.tensor_copy(dw[:], accb[0:1, 2 * G:6 * G])
                    dtmp = fin_pool.tile([1, 2, G], f32, tag="dtmp")
                    nc.vector.tensor_tensor(dtmp[:, 0], dw[:, 0], dw[:, 1],
                                            OP.add)
                    nc.vector.tensor_tensor(dtmp[:, 1], dw[:, 2], dw[:, 3],
                                            OP.add)
                    nc.vector.tensor_tensor(dsb[:, ipl], dtmp[:, 0],
                                            dtmp[:, 1], OP.add)

                    if collective and SPLIT_AR and ipl == 1:
                        # reduce rec+lig now; overlaps the cross loop
                        cc1_in = dram_pool.tile([P + 1, 2 * 2 * G], ccdt,
                                                name="cc1_in")
                        cc1_out = dram_pool.tile([P + 1, 2 * 2 * G], ccdt,
                                                 name="cc1_out",
                                                 addr_space="Shared" if CC_SHARED else "Local")
                        nc.sync.dma_start(cc1_in[0:P, :], accsb[:, 0:2])
                        nc.sync.dma_start(cc1_in[P:P + 1, 0:2 * G], dsb[:, 0:2])
                        nc.gpsimd.collective_compute(
                            "AllReduce", mybir.AluOpType.add,
                            replica_groups=[list(range(ncores))],
                            ins=[cc1_in.opt()], outs=[cc1_out.opt()],
                        )

                # ---------------- all-reduce partials ----------------
                if collective and SPLIT_AR:
                    cc2_in = dram_pool.tile([P + 1, 2 * G], ccdt, name="cc2_in")
                    cc2_out = dram_pool.tile([P + 1, 2 * G], ccdt, name="cc2_out",
                                             addr_space="Shared" if CC_SHARED else "Local")
                    nc.sync.dma_start(cc2_in[0:P, :], accsb[:, 2])
                    nc.sync.dma_start(cc2_in[P:P + 1, 0:G], dsb[:, 2])
                    nc.gpsimd.collective_compute(
                        "AllReduce", mybir.AluOpType.add,
                        replica_groups=[list(range(ncores))],
                        ins=[cc2_in.opt()], outs=[cc2_out.opt()],
                    )
                    red = fin_pool.tile([P, 3, 2, G], ccdt, tag="red")
                    rden = fin_pool.tile([1, 3, G], ccdt, tag="rden")
                    nc.sync.dma_start(red[:, 0:2], cc1_out[0:P, :])
                    nc.sync.dma_start(rden[:, 0:2], cc1_out[P:P + 1, 0:2 * G])
                    nc.sync.dma_start(red[:, 2], cc2_out[0:P, :])
                    nc.sync.dma_start(rden[:, 2], cc2_out[P:P + 1, 0:G])
                elif collective:
                    cc_in = dram_pool.tile([P + 1, 3 * 2 * G], ccdt)
                    cc_out = dram_pool.tile([P + 1, 3 * 2 * G], ccdt,
                                            addr_space="Shared" if CC_SHARED else "Local")
                    nc.sync.dma_start(cc_in[0:P, :], accsb[:])
                    nc.sync.dma_start(cc_in[P:P + 1, 0:3 * G], dsb[:])
                    nc.gpsimd.collective_compute(
                        "AllReduce", mybir.AluOpType.add,
                        replica_groups=[list(range(ncores))],
                        ins=[cc_in.opt()], outs=[cc_out.opt()],
                    )
                    red = fin_pool.tile([P, 3, 2, G], ccdt, tag="red")
                    rden = fin_pool.tile([1, 3, G], ccdt, tag="rden")
                    nc.sync.dma_start(red[:], cc_out[0:P, :])
                    nc.sync.dma_start(rden[:], cc_out[P:P + 1, 0:3 * G])
                else:
                    red = accsb
                    rden = dsb

                # ---------------- pooledT = accT / den ; combT [128, 6, G] ----
                dsum = fin_pool.tile([1, 3, G], f32, tag="dsum")
                nc.vector.tensor_scalar(dsum[:], rden[:], 1e-30, None, OP.add)
                rcp = fin_pool.tile([1, 3, G], f32, tag="rcp")
                nc.vector.reciprocal(rcp[:], dsum[:])
                # broadcast rcp along partitions via K=1 matmul
                rcpb = psum_s.tile([P, 3 * G], f32, tag="s", name="rcpb")
                nc.tensor.matmul(rcpb[:], ones1_sb[:], rcp[0:1, :, :],
                                 start=True, stop=True)
                combT = fin_pool.tile([P, 6, G], f32, tag="combT")
                for k in range(6):
                    nc.vector.tensor_tensor(
                        combT[:, k], red[:, k // 2, k % 2],
                        rcpb[:, (k // 2) * G:(k // 2 + 1) * G], OP.mult)

                # ---------------- energy MLP (fp32, replicated) ----------------
                r1 = fin_pool.tile([P, 2, G], f32, tag="r1")
                for m in range(2):
                    h1p = psum_h.tile([P, G], f32, tag="h")
                    for k in range(6):
                        nc.tensor.matmul(h1p[:], mw1_sb[:, k, m * P:(m + 1) * P],
                                         combT[:, k], start=(k == 0), stop=(k == 5))
                    nc.scalar.activation(r1[:, m], h1p[:], AF.Relu,
                                         bias=mb1_sb[:, m:m + 1])
                h2p = psum_h.tile([P, G], f32, tag="h")
                for m in range(2):
                    nc.tensor.matmul(h2p[:], mw2_sb[:, m], r1[:, m],
                                     start=(m == 0), stop=(m == 1))
                r2 = fin_pool.tile([P, G], f32, tag="r2")
                nc.scalar.activation(r2[:], h2p[:], AF.Relu, bias=mb2_sb[:])

                ep = psum_s.tile([1, G], f32, tag="s", name="ep")
                nc.tensor.matmul(ep[:], ow_sb[:], r2[:], start=True, stop=True)
                en = fin_pool.tile([1, G], f32, tag="en")
                nc.vector.tensor_scalar(en[:], ep[:], ob_sb[:], None, OP.add)
                nc.sync.dma_start(energyd.ap(), en[:])

    nc.compile()
    nc.m = get_hw_module(nc.m)
    return nc


def _prep_pool(x, idx, n_pad):
    """Shard rows of x/idx across cores, pad to n_pad rows per core.
    Returns per-core lists (x [n_pad,256] bf16, idx [128, n_pad//128] bf16)."""
    import ml_dtypes
    n = x.shape[0]
    per = math.ceil(n / NCORES)
    xs, idxs = [], []
    for c in range(NCORES):
        lo, hi = c * per, min((c + 1) * per, n)
        xp = np.zeros((n_pad, FOLD), dtype=ml_dtypes.bfloat16)
        xp[:hi - lo] = x[lo:hi].astype(ml_dtypes.bfloat16)
        ip = np.full((n_pad,), -1.0, dtype=np.float32)
        ip[:hi - lo] = idx[lo:hi]
        # rearrange (g n p) -> p (g n)
        ip = ip.reshape(n_pad // RG, NPG, P).transpose(2, 0, 1)
        xs.append(xp)
        idxs.append(np.ascontiguousarray(ip.reshape(P, -1)))
    return xs, idxs


def kernel(rec_na, lig_na, cross_ea, cross_idx, protein_batch, ligand_batch,
           num_graphs,
           pa_W1, pa_b1, pa_W2, pa_b2,
           la_W1, la_b1, la_W2, la_b2,
           ca_W1, ca_b1, ca_W2, ca_b2,
           mlp_W1, mlp_b1, mlp_W2, mlp_b2,
           out_W, out_b):
    import ml_dtypes
    from concourse import bass_utils

    bf = ml_dtypes.bfloat16
    assert int(num_graphs) == G
    rec_na = np.asarray(rec_na, dtype=np.float32)
    lig_na = np.asarray(lig_na, dtype=np.float32)
    cross_ea = np.asarray(cross_ea, dtype=np.float32)
    cross_idx = np.asarray(cross_idx)
    protein_batch = np.asarray(protein_batch)
    ligand_batch = np.asarray(ligand_batch)
    complex_ids = ligand_batch[cross_idx[0]]

    data = {
        "rec": (rec_na, protein_batch.astype(np.float32),
                pa_W1, pa_b1, pa_W2, pa_b2),
        "lig": (lig_na, ligand_batch.astype(np.float32),
                la_W1, la_b1, la_W2, la_b2),
        "cross": (cross_ea, complex_ids.astype(np.float32),
                  ca_W1, ca_b1, ca_W2, ca_b2),
    }
    cfg = {pl: _pad_rows(data[pl][0].shape[0]) for pl in _POOLS}

    key = tuple(sorted(cfg.items()))
    if key not in _cache:
        _cache[key] = _build(cfg)
    nc = _cache[key]

    shared = {
        "iota": np.broadcast_to(np.arange(G, dtype=np.float32),
                                (P, G)).astype(bf),
        "ident": np.eye(P, dtype=np.float32).astype(bf),
        "identf": np.eye(P, dtype=np.float32),
        "ones_col": np.ones((P, 2), dtype=np.float32).astype(bf),
        "ones_row": np.ones((1, P), dtype=np.float32),
        "mlp_w1": np.asarray(mlp_W1, dtype=np.float32),
        "mlp_b1": np.ascontiguousarray(
            np.asarray(mlp_b1, dtype=np.float32).reshape(2, P).T),
        "mlp_w2": np.asarray(mlp_W2, dtype=np.float32),
        "mlp_b2": np.asarray(mlp_b2, dtype=np.float32).reshape(P, 1),
        "out_w": np.asarray(out_W, dtype=np.float32),
        "out_b": np.asarray(out_b, dtype=np.float32).reshape(1, 1),
    }
    percore = [dict(shared) for _ in range(NCORES)]
    for pl in _POOLS:
        x, idx, W1, b1, W2, b2 = data[pl]
        xs, idxs = _prep_pool(x, idx, cfg[pl])
        W1 = np.asarray(W1, dtype=np.float32)
        if TPOSE in ("packed", "packedr"):
            W1 = np.concatenate([W1[0::2], W1[1::2]])
        W1 = W1.astype(bf)
        W2 = np.concatenate([np.asarray(W2, np.float32).reshape(HALF, 1),
                             np.zeros((HALF, 1), np.float32)],
                            axis=1).astype(bf)
        b1 = np.asarray(b1, dtype=np.float32).reshape(HALF, 1)
        b2 = np.broadcast_to(np.asarray(b2, dtype=np.float32).reshape(1, 1),
                             (P, 1)).copy()
        for c in range(NCORES):
            percore[c][f"x_{pl}"] = (xs[c].view(np.float32)
                                     if TPOSE == "packedr" else xs[c])
            percore[c][f"idx_{pl}"] = idxs[c]
            percore[c][f"w1_{pl}"] = W1
            percore[c][f"w2_{pl}"] = W2
            percore[c][f"b1_{pl}"] = b1
            percore[c][f"b2_{pl}"] = b2

    global _LAST
    _LAST = (nc, percore)
    if _PREP_ONLY:
        return None
    res = bass_utils.run_bass_kernel_spmd(nc, percore,
                                          core_ids=list(range(NCORES)))
    return np.asarray(res.results[0]["energy"], dtype=np.float32)


def prepare(inputs):
    """Build (or fetch cached) program + per-core input maps without running."""
    global _PREP_ONLY
    _PREP_ONLY = True
    try:
        kernel(**inputs)
    finally:
        _PREP_ONLY = False
    return _LAST


_PREP_ONLY = False
_LAST = None
